# revision 1
# baseline (speedup 1.0000x reference)
"""DigitCapsule dynamic-routing kernel for 8 TRN2 NeuronCores.

Strategy: the reference routing is fully independent per output capsule c
(softmax over routes, sums over routes, batch-mean are all per-c). So we
shard the C=64 capsules 8-ways: each core gets W[:, 8k:8k+8] (8 MiB) and a
replicated x (4 MiB). Zero collectives; identical SPMD program per core with
per-core inputs.

Per core (B=64, R=2048, I=8, CL=8, O=16; K-dim = (r,i) = 16384 = 128 k-tiles):
  pass 0:  s0^T[(c,o),b] = sum_t Wr_t^T @ XT_t          (c_ij uniform = 1/R)
  iter 1,2:
    G[(r,i),(c,o)]  = X^T-slices @ V  (per k-tile, K=b=64)
    P = G (.) Wr   ->  BD-matmul sums i ->  reduce o  ->  ab[r,c] (batch-mean)
    b += ab/B ;  w = exp(b)  (softmax sans max-shift: |b| < 0.2)
    WW_t = Wr_t * w[r,c] (broadcast over i,o);  N^T = sum_t WW_t^T @ XT_t
    Z[c] = sum_r w[r,c];  s = N^T / Z;  v = squash(s)
  out = v^T  -> [b,(c,o)]
"""

import os
import sys

for _p in ("/opt/trn_rl_repo", "/root/.axon_site/_ro/trn_rl_repo"):
    if os.path.isdir(_p) and _p not in sys.path:
        sys.path.insert(0, _p)

from contextlib import ExitStack

import numpy as np

import concourse.bass as bass
import concourse.bacc as bacc
from concourse import mybir
from concourse.bass_utils import run_bass_kernel_spmd
from concourse.tile import TileContext

B, R, C, O, I = 64, 2048, 64, 16, 8
N_CORES = 8
CL = C // N_CORES            # capsules per core = 8
F = CL * O                   # free (c,o) = 128
NT = R // 16                 # 128 k-tiles; tile t = routes [16t,16t+16), part p=(q,i)
NH = 16                      # number of G/P blocks (8 k-tiles each)
BLK = NT // NH               # 8 k-tiles per block
EPS = 1e-8

COMPUTE = os.environ.get("CAPS_DTYPE", "bf16")  # "bf16" | "f32"
STAGE = int(os.environ.get("CAPS_STAGE", "2"))   # 0: pass0 only; 1: one iter; 2: full
SUB = os.environ.get("CAPS_SUB", "full")         # gpath | bpath | full

# consts columns (f32): BDF4 [0:512), RepM [512:640), ident [640:768),
# ones16 [768:769), RepC [769:897), Sel4 [897:961)
CW = 961


def _consts_np():
    cst = np.zeros((128, CW), dtype=np.float32)
    p = np.arange(128)
    # BDF4_j[p, m] = 1 iff m == 32j + p//8 (i-reduce into row band 32j)
    for j in range(4):
        cst[p, 128 * j + 32 * j + p // 8] = 1.0
    # RepM[k=q (rows 0..15), m=(q',i)] = 1 iff m//8 == q
    for q in range(16):
        cst[q, 512 + q * 8:512 + q * 8 + 8] = 1.0
    # ident[p, f] = 1 iff p == f
    cst[p, 640 + p] = 1.0
    # ones16[p] = 1 for p < 16 (Z reduction over q-rows)
    cst[0:16, 768] = 1.0
    # RepC[k=c(rows 0..7), m=(o,c')] = 1 iff m%8 == c
    for c in range(8):
        cst[c, 769 + c:897:8] = 1.0
    # Sel4_j[p, m] = 1 iff p == 32j + m (m < 16): extract row band 32j
    for j in range(4):
        for m in range(16):
            cst[32 * j + m, 897 + 16 * j + m] = 1.0
    return cst


def _squash(nc, pool, s_sb, Pdim=128, Nfree=64):
    """v = s^2*s / ((1+s^2)*|s| + EPS), elementwise on [Pdim, Nfree] f32."""
    f32 = mybir.dt.float32
    sq = pool.tile([Pdim, Nfree], f32, tag="sq", name="sq")
    rt = pool.tile([Pdim, Nfree], f32, tag="rt", name="rt")
    num = pool.tile([Pdim, Nfree], f32, tag="num", name="num")
    den = pool.tile([Pdim, Nfree], f32, tag="den", name="den")
    rd = pool.tile([Pdim, Nfree], f32, tag="rd", name="rd")
    v = pool.tile([Pdim, Nfree], f32, tag="v", name="v")
    nc.scalar.activation(sq, s_sb, mybir.ActivationFunctionType.Square)
    nc.scalar.activation(rt, sq, mybir.ActivationFunctionType.Sqrt)
    nc.vector.tensor_mul(num, sq, s_sb)
    nc.vector.scalar_tensor_tensor(den, sq, 1.0, rt,
                                   op0=mybir.AluOpType.add,
                                   op1=mybir.AluOpType.mult)
    nc.vector.tensor_scalar_add(den, den, EPS)
    nc.vector.reciprocal(rd, den)
    nc.vector.tensor_mul(v, num, rd)
    return v


def build_bass(compute=COMPUTE):
    f32 = mybir.dt.float32
    cdt = mybir.dt.bfloat16 if compute == "bf16" else f32
    PBUFS = 3 if compute == "bf16" else 1
    SBUFS = 2 if compute == "bf16" else 1
    WWB = 6 if compute == "bf16" else 3

    nc = bacc.Bacc()
    # wxt: 8 chunks of [wk 2048 | xt 1024 | xn 2048] columns
    wxt_d = nc.declare_dram_parameter("wxt", [128, 8 * 5120], cdt, isOutput=False)
    cst_d = nc.declare_dram_parameter("consts", [128, CW], f32, isOutput=False)
    out_d = nc.declare_dram_parameter("out", [B, F], f32, isOutput=True)

    with TileContext(nc) as tc, ExitStack() as ctx:
        big = ctx.enter_context(tc.tile_pool(name="big", bufs=1))
        small = ctx.enter_context(tc.tile_pool(name="small", bufs=3))
        sq_pool = ctx.enter_context(tc.tile_pool(name="sqp", bufs=3))
        ppool = ctx.enter_context(tc.tile_pool(name="ppool", bufs=2))
        wwpool = ctx.enter_context(tc.tile_pool(name="wwpool", bufs=3))
        ps_acc = ctx.enter_context(tc.tile_pool(name="ps_acc", bufs=1, space="PSUM"))
        ps_g = ctx.enter_context(tc.tile_pool(name="ps_g", bufs=2, space="PSUM"))
        ps_bd = ctx.enter_context(tc.tile_pool(name="ps_bd", bufs=1, space="PSUM"))
        ps_misc = ctx.enter_context(tc.tile_pool(name="ps_misc", bufs=1, space="PSUM"))

        # ---- load inputs (one DMA per chunk => single wait per consumer) ----
        wxt = [big.tile([128, 5120], cdt, tag=f"wxt{h}", name=f"wxt{h}")
               for h in range(8)]
        # split each chunk's wk+xt DMA 4 ways so early chunks land first
        for h in range(8):
            for piece in range(4):
                c0 = piece * 768
                nc.sync.dma_start(
                    out=wxt[h][:, c0:c0 + 768],
                    in_=wxt_d[:, h * 5120 + c0:h * 5120 + c0 + 768])
        for h in range(8):
            nc.sync.dma_start(out=wxt[h][0:64, 3072:5120],
                              in_=wxt_d[0:64, h * 5120 + 3072:(h + 1) * 5120])
        cst_raw = big.tile([128, CW], f32, tag="cst_raw", name="cst_raw")
        nc.sync.dma_start(out=cst_raw, in_=cst_d[:])
        cst = big.tile([128, CW], f32, tag="cst", name="cst")
        nc.vector.tensor_copy(cst, cst_raw)
        BDF4c = big.tile([128, 512], cdt, tag="bdf4c", name="bdf4c")
        nc.vector.tensor_copy(BDF4c, cst[:, 0:512])
        RepM = cst[0:16, 512:640]
        RepMc = big.tile([16, 128], cdt, tag="repmc", name="repmc")
        nc.vector.tensor_copy(RepMc, RepM)
        ident = cst[:, 640:768]
        ones16 = cst[0:16, 768:769]
        RepC = cst[0:8, 769:897]
        Sel4 = cst[:, 897:961]

        def wk_tile(t):
            h, lo = t // 16, t % 16
            return wxt[h][:, lo * 128:(lo + 1) * 128]

        def xt_tile(t):
            h, lo = t // 16, t % 16
            return wxt[h][:, 2048 + lo * 64:2048 + (lo + 1) * 64]

        def xn_col(t):
            h, lo = t // 16, t % 16
            return wxt[h][0:64, 3072 + lo * 128:3072 + (lo + 1) * 128]

        # ---- pass 0: s0T = sum_t Wr_t^T @ XT_t ----
        ps_s = ps_acc.tile([128, 64], f32, tag="acc", name="acc")
        for t in range(NT):
            nc.tensor.matmul(ps_s, lhsT=wk_tile(t), rhs=xt_tile(t),
                             start=(t == 0), stop=(t == NT - 1))
        s_sb = small.tile([128, 64], f32, tag="s", name="s")
        nc.vector.tensor_scalar_mul(s_sb, ps_s, 1.0 / R)
        vT = _squash(nc, sq_pool, s_sb)

        # V: [64,128] cdt — v^T transposed to [b,(c,o)]
        def make_V(vT_sb):
            ps_t = ps_misc.tile([64, 128], f32, tag="misc", name="misc")
            nc.tensor.transpose(ps_t, vT_sb, ident)
            v2 = small.tile([64, 128], cdt, tag="v2", name="v2", bufs=2)
            nc.vector.tensor_copy(v2, ps_t)
            return v2

        V2 = make_V(vT)

        bstate = small.tile([16, NT * 8], f32, tag="bstate", name="bstate", bufs=1)
        nc.vector.memset(bstate, 0.0)

        iters = [1, 2][:STAGE]
        for it in iters:
            # ---------- a-path: ab[r,c] = mean_b <u_hat, v> ----------
            ab = small.tile([16, NT * 8], f32, tag="ab", name="ab", bufs=SBUFS)
            wexp = small.tile([16, NT * 8], f32, tag="wexp", name="wexp", bufs=SBUFS)
            wexpb = small.tile([16, NT * 8], cdt, tag="wexpb", name="wexpb", bufs=SBUFS)
            wreps = [None] * 8
            ps_n = ps_acc.tile([128, 64], f32, tag="acc", name="acc")
            for grp in range(4):          # 4 groups x 4 blocks x 8 k-tiles
                psb = ps_bd.tile([128, BLK * 128], f32, tag="bd", name="bd")
                for j in range(4):
                    h = 4 * grp + j
                    psg = ps_g.tile([128, BLK * 128], f32, tag="g", name="g")
                    for lo in range(BLK):
                        t = h * BLK + lo
                        nc.tensor.matmul(
                            psg[:, lo * 128:(lo + 1) * 128],
                            lhsT=xn_col(t),
                            rhs=V2,
                            start=True, stop=True,
                        )
                    Pg = ppool.tile([128, BLK * 128], cdt, tag="Pg", name="Pg",
                                    bufs=PBUFS)
                    nc.scalar.activation(Pg, psg,
                                         mybir.ActivationFunctionType.Copy)
                    P = ppool.tile([128, BLK * 128], cdt, tag="P", name="P",
                                   bufs=PBUFS)
                    wkh = wxt[h // 2][:, 0:2048].rearrange("p (u f) -> p u f",
                                                           f=128)
                    nc.vector.tensor_mul(
                        P.rearrange("p (u f) -> p u f", f=128),
                        Pg.rearrange("p (u f) -> p u f", f=128),
                        wkh[:, (h % 2) * BLK:(h % 2) * BLK + BLK, :],
                    )
                    for half in range(2):
                        nc.tensor.matmul(
                            psb[:, half * 512:(half + 1) * 512],
                            lhsT=BDF4c[:, 128 * j:128 * (j + 1)],
                            rhs=P[:, half * 512:(half + 1) * 512],
                            start=(j == 0), stop=(j == 3),
                        )
                # reduce o on packed bands -> [128, (lo, c)]
                ored = small.tile([128, 64], f32, tag="ored", name="ored",
                                  bufs=3)
                nc.vector.tensor_reduce(
                    ored.rearrange("p (lo c) -> p lo c", c=8),
                    bass.AP(tensor=psb.tensor, offset=psb.offset,
                            ap=[psb.ap[0], [128, 8], [1, 8], [8, 16]]),
                    axis=mybir.AxisListType.X,
                    op=mybir.AluOpType.add,
                )
                # re-base bands to rows 0..15 and pack into ab columns
                ps_sel = ps_misc.tile([16, 256], f32, tag="misc", name="misc")
                for j in range(4):
                    nc.tensor.matmul(ps_sel[:, 64 * j:64 * (j + 1)],
                                     lhsT=Sel4[:, 16 * j:16 * (j + 1)],
                                     rhs=ored,
                                     start=True, stop=True)
                nc.vector.tensor_copy(ab[:, grp * 256:(grp + 1) * 256], ps_sel)
            # per-group: b-update, exp, wrep, then WW (.) + N matmuls for
            # this group's 32 k-tiles — overlaps a-path of next group
            for grp in range(4):
                cs = slice(grp * 256, (grp + 1) * 256)
                nc.vector.scalar_tensor_tensor(bstate[:, cs], ab[:, cs], 1.0 / B,
                                               bstate[:, cs],
                                               op0=mybir.AluOpType.mult,
                                               op1=mybir.AluOpType.add)
                nc.scalar.activation(wexp[:, cs], bstate[:, cs],
                                     mybir.ActivationFunctionType.Exp)
                nc.vector.tensor_copy(wexpb[:, cs], wexp[:, cs])
                for uu in range(2):
                    u = 2 * grp + uu
                    ps_w = ps_misc.tile([128, 128], f32, tag="misc", name="wrps")
                    nc.tensor.matmul(ps_w, lhsT=RepMc,
                                     rhs=wexpb[:, u * 128:(u + 1) * 128],
                                     start=True, stop=True)
                    wr = small.tile([128, 128], cdt, tag=f"wrs{u}", name="wrs",
                                    bufs=SBUFS)
                    nc.scalar.activation(wr, ps_w,
                                         mybir.ActivationFunctionType.Copy)
                    wreps[u] = wr
                for q4 in range(8 * grp, 8 * grp + 8):
                    ww = wwpool.tile([128, 4 * 128], cdt, tag="ww", name="ww", bufs=WWB)
                    h8 = (4 * q4) // 16
                    lo0 = (4 * q4) % 16
                    wkc = wxt[h8][:, lo0 * 128:(lo0 + 4) * 128]
                    wr = wreps[q4 // 4]
                    tc0 = (4 * q4) % 16
                    in1 = bass.AP(
                        tensor=wr.tensor,
                        offset=wr[:, tc0 * 8:tc0 * 8 + 1].offset,
                        ap=[wr.ap[0], [8, 4], [0, 16], [1, 8]],
                    )
                    nc.vector.tensor_tensor(
                        ww.rearrange("p (g o c) -> p g o c", o=16, c=8),
                        wkc.rearrange("p (g o c) -> p g o c", o=16, c=8),
                        in1,
                        op=mybir.AluOpType.mult,
                    )
                    for gl in range(4):
                        t = 4 * q4 + gl
                        nc.tensor.matmul(ps_n,
                                         lhsT=ww[:, gl * 128:(gl + 1) * 128],
                                         rhs=xt_tile(t),
                                         start=(t == 0), stop=(t == NT - 1))

            # Z[c] (needs full wexp) -> rz -> rzrep
            wsum = small.tile([16, 8], f32, tag="wsum", name="wsum")
            nc.vector.tensor_reduce(
                wsum,
                bass.AP(tensor=wexp.tensor, offset=wexp.offset,
                        ap=[wexp.ap[0], [1, 8], [8, NT]]),
                axis=mybir.AxisListType.X, op=mybir.AluOpType.add,
            )
            ps_z = ps_misc.tile([8, 1], f32, tag="misc", name="misc")
            nc.tensor.matmul(ps_z, lhsT=wsum, rhs=ones16, start=True, stop=True)
            rz = small.tile([8, 1], f32, tag="rz", name="rz")
            nc.vector.reciprocal(rz, ps_z)
            ps_rz = ps_misc.tile([128, 1], f32, tag="misc", name="misc")
            nc.tensor.matmul(ps_rz, lhsT=RepC, rhs=rz, start=True, stop=True)
            rzrep = small.tile([128, 1], f32, tag="rzrep", name="rzrep")
            nc.vector.tensor_copy(rzrep, ps_rz)
            # s = N^T * rzrep ; v = squash(s)
            s_it = small.tile([128, 64], f32, tag="s", name="s")
            nc.vector.tensor_scalar_mul(s_it, ps_n, rzrep)
            vT = _squash(nc, sq_pool, s_it)
            if it < 2:
                V2 = make_V(vT)

        # ---- output: out[b,(c,o)] = vT^T ----
        ps_o = ps_misc.tile([64, 128], f32, tag="misc", name="misc")
        nc.tensor.transpose(ps_o, vT, ident)
        out_sb = small.tile([64, 128], f32, tag="outsb", name="outsb")
        nc.vector.tensor_copy(out_sb, ps_o)
        nc.sync.dma_start(out=out_d[:], in_=out_sb)

    nc.finalize()
    return nc


def _host_prep(x, W, compute=COMPUTE):
    """Build per-core input dicts."""
    if compute == "bf16":
        import ml_dtypes
        ct = ml_dtypes.bfloat16
    else:
        ct = np.float32
    x = np.ascontiguousarray(x, dtype=np.float32)
    W = np.ascontiguousarray(W, dtype=np.float32)
    # xt[p=(q,i), t*64+b] = x[b, 16t+q, i]
    xt = x.reshape(B, NT, 16, I).transpose(2, 3, 1, 0).reshape(128, NT, 64)
    # xn[b, t*128 + q*8+i] = x[b, 16t+q, i]  (natural layout, partitions 0..63)
    xn = x.reshape(B, NT, 128)
    cst = _consts_np()
    in_maps = []
    for k in range(N_CORES):
        Ws = W[:, k * CL:(k + 1) * CL]  # [R, 8, O, I]
        wk = (Ws.reshape(NT, 16, CL, O, I).transpose(1, 4, 0, 3, 2)
              .reshape(128, NT, 128))
        wxt = np.zeros((128, 8, 5120), dtype=np.float32)
        for h in range(8):
            wxt[:, h, 0:2048] = wk[:, 16 * h:16 * (h + 1), :].reshape(128, 2048)
            wxt[:, h, 2048:3072] = xt[:, 16 * h:16 * (h + 1), :].reshape(128, 1024)
            wxt[0:64, h, 3072:5120] = xn[:, 16 * h:16 * (h + 1), :].reshape(64, 2048)
        in_maps.append({
            "wxt": np.ascontiguousarray(wxt.reshape(128, 8 * 5120), dtype=ct),
            "consts": cst,
        })
    return in_maps


_CACHE = {}


def _get_nc(compute=COMPUTE):
    if compute not in _CACHE:
        _CACHE[compute] = build_bass(compute)
    return _CACHE[compute]


def run(x, W, compute=COMPUTE, trace=False):
    nc = _get_nc(compute)
    in_maps = _host_prep(x, W, compute)
    res = run_bass_kernel_spmd(nc, in_maps, core_ids=list(range(N_CORES)),
                               trace=trace)
    outs = [np.asarray(res.results[k]["out"], dtype=np.float32)
            for k in range(N_CORES)]
    # out[b, (o, c)]: core k holds capsules [8k, 8k+8)
    v = np.concatenate(
        [o.reshape(B, O, CL).transpose(0, 2, 1) for o in outs], axis=1)
    return v[..., None], res


def kernel(x, W):
    v, _ = run(np.asarray(x), np.asarray(W))
    return v



# revision 3
# speedup vs baseline: 1.0821x; 1.0821x over previous
"""DigitCapsule dynamic-routing kernel for 8 TRN2 NeuronCores.

Strategy: the reference routing is fully independent per output capsule c
(softmax over routes, sums over routes, batch-mean are all per-c). So we
shard the C=64 capsules 8-ways: each core gets W[:, 8k:8k+8] and a
replicated x. Zero collectives; identical SPMD program per core with
per-core inputs.

Per core (B=64, R=2048, I=8, CL=8, O=16; K-dim = (r,i) = 16384 = 128
k-tiles of 128 = (16 routes q, 8 i)). All s/v tensors live as
[b=64, (o,c)=128]; routing state lives banded as [(j,q)=128, (g,lo,c)].

  pass 0:  n0[b,(o,c)] = sum_t xt_t^T @ wk_t          (c_ij uniform)
           v = n|n| / (R^2 + n^2)       == squash(n/R), exact algebra
  iter 1,2:
    per 8-k-tile block hb (16 blocks):
      G[(q,i),(lo,(o,c))] = xn_col(t)^T @ V            (8 matmuls)
      P = G (.) Wr                                      (bf16 mul)
      BD-matmul accumulates bands: psb[(j,q),(lo,o,c)] per grp of 4 blocks
    per grp: ored = reduce_o(psb) -> [128,(lo,c)]; bstate += ored/B
      wexpb = exp(bstate-slice) (softmax numerator; |b| small, no shift)
      wr[(q,i),(lo,c)] = band-broadcast(wexpb)          (matmul w/ BDT_j)
      WW = Wr (.) wr (broadcast o);  n += xt_t^T @ WW_t (8 matmuls/blk)
    Z[c] = sum_r wexp;  v = n|n| / (Z^2 + n^2)  == squash(n/Z), exact
  out[b,(o,c)] = v (f32)
"""

import os
import sys

for _p in ("/opt/trn_rl_repo", "/root/.axon_site/_ro/trn_rl_repo"):
    if os.path.isdir(_p) and _p not in sys.path:
        sys.path.insert(0, _p)

from contextlib import ExitStack

import numpy as np

import concourse.bass as bass
import concourse.bacc as bacc
from concourse import mybir
from concourse.bass_utils import run_bass_kernel_spmd
from concourse.tile import TileContext

B, R, C, O, I = 64, 2048, 64, 16, 8
N_CORES = 8
CL = C // N_CORES            # capsules per core = 8
F = CL * O                   # free (o,c) = 128
NT = R // 16                 # 128 k-tiles; tile t = routes [16t,16t+16), part p=(q,i)
NB = 16                      # number of 8-k-tile blocks
BLK = NT // NB               # 8 k-tiles per block

# how many of the 16 P/WW multiplies per iter go to GpSimd instead of DVE
GPS_P = int(os.environ.get("CAPS_GPS_P", "0"))
GPS_WW = int(os.environ.get("CAPS_GPS_WW", "0"))


def _consts_np():
    """cstb [128,1024] bf16: BDF4 [0:512), BDT [512:1024).
    cstf [128,65] f32: ones128 col 0; ones-row (partition 0) cols [1:65)."""
    cstb = np.zeros((128, 1024), dtype=np.float32)
    p = np.arange(128)
    # BDF4_j[p=(q,i), m] = 1 iff m == 32j + p//8  (i-reduce into band 32j+q)
    for j in range(4):
        cstb[p, 128 * j + 32 * j + p // 8] = 1.0
    # BDT_j = BDF4_j^T (band (j,q) -> rows (q,i))
    for j in range(4):
        cstb[:, 512 + 128 * j:512 + 128 * (j + 1)] = \
            cstb[:, 128 * j:128 * (j + 1)].T
    cstf = np.zeros((128, 65), dtype=np.float32)
    # Z-reduce mask: only band rows 32j+q (q<16) hold real data; the other
    # 64 partitions of wexpb are exp(0)=1 junk and must not enter Z.
    cstf[p[(p % 32) < 16], 0] = 1.0
    cstf[0, 1:65] = 1.0
    return cstb, cstf


def build_bass():
    f32 = mybir.dt.float32
    cdt = mybir.dt.bfloat16

    nc = bacc.Bacc()
    # wxt: 8 chunks of [wk 2048 | xt 1024 | xn 2048] columns
    wxt_d = nc.declare_dram_parameter("wxt", [128, 8 * 5120], cdt, isOutput=False)
    cstb_d = nc.declare_dram_parameter("cstb", [128, 1024], cdt, isOutput=False)
    cstf_d = nc.declare_dram_parameter("cstf", [128, 65], f32, isOutput=False)
    out_d = nc.declare_dram_parameter("out", [B, F], f32, isOutput=True)

    with TileContext(nc) as tc, ExitStack() as ctx:
        big = ctx.enter_context(tc.tile_pool(name="big", bufs=1))
        small = ctx.enter_context(tc.tile_pool(name="small", bufs=3))
        ppool = ctx.enter_context(tc.tile_pool(name="ppool", bufs=3))
        wwpool = ctx.enter_context(tc.tile_pool(name="wwpool", bufs=3))
        ps_acc = ctx.enter_context(tc.tile_pool(name="ps_acc", bufs=1, space="PSUM"))
        ps_g = ctx.enter_context(tc.tile_pool(name="ps_g", bufs=2, space="PSUM"))
        ps_bd = ctx.enter_context(tc.tile_pool(name="ps_bd", bufs=1, space="PSUM"))
        ps_misc = ctx.enter_context(tc.tile_pool(name="ps_misc", bufs=1, space="PSUM"))

        # ---- load inputs (consts first: they're small and needed early) ----
        cstb = big.tile([128, 1024], cdt, tag="cstb", name="cstb")
        nc.sync.dma_start(out=cstb, in_=cstb_d[:])
        cstf = big.tile([128, 65], f32, tag="cstf", name="cstf")
        nc.sync.dma_start(out=cstf, in_=cstf_d[:])
        wxt = [big.tile([128, 5120], cdt, tag=f"wxt{h}", name=f"wxt{h}")
               for h in range(8)]
        # split each chunk's wk+xt DMA 4 ways so early chunks land first
        for h in range(8):
            for piece in range(4):
                c0 = piece * 768
                nc.sync.dma_start(
                    out=wxt[h][:, c0:c0 + 768],
                    in_=wxt_d[:, h * 5120 + c0:h * 5120 + c0 + 768])
        for h in range(8):
            nc.sync.dma_start(out=wxt[h][0:64, 3072:5120],
                              in_=wxt_d[0:64, h * 5120 + 3072:(h + 1) * 5120])

        BDF4 = cstb[:, 0:512]
        BDT = cstb[:, 512:1024]
        ones128 = cstf[:, 0:1]
        onesrow = cstf[0:1, 1:65]

        def wk_tile(t):
            h, lo = t // 16, t % 16
            return wxt[h][:, lo * 128:(lo + 1) * 128]

        def xt_tile(t):
            h, lo = t // 16, t % 16
            return wxt[h][:, 2048 + lo * 64:2048 + (lo + 1) * 64]

        def xn_col(t):
            h, lo = t // 16, t % 16
            return wxt[h][0:64, 3072 + lo * 128:3072 + (lo + 1) * 128]

        def wk_block(hb):
            # [128, 8, 128] view of block hb's 8 k-tiles of W
            wkh = wxt[hb // 2][:, 0:2048].rearrange("p (u f) -> p u f", f=128)
            return wkh[:, (hb % 2) * BLK:(hb % 2) * BLK + BLK, :]

        # v = n*|n| / (zsq + n^2); written to dst (V bf16 or out f32)
        def squash_from(ps_n, zsq_sb, dst):
            absn = small.tile([64, 128], f32, tag="absn", name="absn")
            nc.scalar.activation(absn, ps_n, mybir.ActivationFunctionType.Abs)
            nsq = small.tile([64, 128], f32, tag="nsq", name="nsq")
            nc.scalar.activation(nsq, ps_n, mybir.ActivationFunctionType.Square)
            den = small.tile([64, 128], f32, tag="den", name="den")
            if zsq_sb is None:
                nc.vector.tensor_scalar_add(den, nsq, float(R) * float(R))
            else:
                nc.vector.tensor_add(den, nsq, zsq_sb)
            rden = small.tile([64, 128], f32, tag="rden", name="rden")
            nc.vector.reciprocal(rden, den)
            num = small.tile([64, 128], f32, tag="num", name="num")
            nc.vector.tensor_mul(num, ps_n, absn)
            nc.vector.tensor_mul(dst, num, rden)

        # ---- pass 0: n0 = sum_t xt_t^T @ wk_t ; V = squash ----
        ps_s = ps_acc.tile([64, 128], f32, tag="acc", name="acc")
        for t in range(NT):
            nc.tensor.matmul(ps_s, lhsT=xt_tile(t), rhs=wk_tile(t),
                             start=(t == 0), stop=(t == NT - 1))
        V = small.tile([64, 128], cdt, tag="V", name="V", bufs=2)
        squash_from(ps_s, None, V)

        bstate = small.tile([128, 256], f32, tag="bstate", name="bstate", bufs=1)
        nc.vector.memset(bstate, 0.0)
        wexpb = small.tile([128, 256], cdt, tag="wexpb", name="wexpb", bufs=1)

        for it in (1, 2):
            ps_n = ps_acc.tile([64, 128], f32, tag="acc", name="acc")
            wrs = [None] * NB
            for grp in range(4):
                # ---------- a-path for this grp's 4 blocks ----------
                psb = ps_bd.tile([128, BLK * 128], f32, tag="bd", name="bd")
                for j in range(4):
                    hb = 4 * grp + j
                    psg = ps_g.tile([128, BLK * 128], f32, tag="g", name="g")
                    for lo in range(BLK):
                        t = hb * BLK + lo
                        nc.tensor.matmul(
                            psg[:, lo * 128:(lo + 1) * 128],
                            lhsT=xn_col(t), rhs=V,
                            start=True, stop=True,
                        )
                    Pg = ppool.tile([128, BLK * 128], cdt, tag="Pg", name="Pg")
                    nc.scalar.activation(Pg, psg,
                                         mybir.ActivationFunctionType.Copy)
                    P = ppool.tile([128, BLK * 128], cdt, tag="P", name="P")
                    eng = nc.gpsimd if hb % 16 < GPS_P else nc.vector
                    eng.tensor_tensor(
                        P.rearrange("p (u f) -> p u f", f=128),
                        Pg.rearrange("p (u f) -> p u f", f=128),
                        wk_block(hb),
                        op=mybir.AluOpType.mult,
                    )
                    for half in range(2):
                        nc.tensor.matmul(
                            psb[:, half * 512:(half + 1) * 512],
                            lhsT=BDF4[:, 128 * j:128 * (j + 1)],
                            rhs=P[:, half * 512:(half + 1) * 512],
                            start=(j == 0), stop=(j == 3),
                        )
                # reduce o: psb [(j,q), (lo,o,c)] -> ored [(j,q), (lo,c)]
                ored = small.tile([128, 64], f32, tag="ored", name="ored",
                                  bufs=2)
                nc.vector.tensor_reduce(
                    ored.rearrange("p (l c) -> p l c", c=8),
                    bass.AP(tensor=psb.tensor, offset=psb.offset,
                            ap=[psb.ap[0], [128, 8], [1, 8], [8, 16]]),
                    axis=mybir.AxisListType.X,
                    op=mybir.AluOpType.add,
                )
                # ---------- b-update + wexp + wrep + WW + n-matmuls ----------
                cs = slice(grp * 64, (grp + 1) * 64)
                nc.vector.scalar_tensor_tensor(bstate[:, cs], ored, 1.0 / B,
                                               bstate[:, cs],
                                               op0=mybir.AluOpType.mult,
                                               op1=mybir.AluOpType.add)
                nc.scalar.activation(wexpb[:, cs], bstate[:, cs],
                                     mybir.ActivationFunctionType.Exp)
                for j in range(4):
                    hb = 4 * grp + j
                    ps_wr = ps_misc.tile([128, 64], f32, tag="m", name="wrps")
                    nc.tensor.matmul(ps_wr, lhsT=BDT[:, 128 * j:128 * (j + 1)],
                                     rhs=wexpb[:, cs], start=True, stop=True)
                    wr = small.tile([128, 64], cdt, tag="wr", name="wr", bufs=3)
                    nc.scalar.activation(wr, ps_wr,
                                         mybir.ActivationFunctionType.Copy)
                    wrs[hb] = wr
                for j in range(4):
                    hb = 4 * grp + j
                    wr = wrs[hb]
                    ww = wwpool.tile([128, BLK * 128], cdt, tag="ww", name="ww")
                    in1 = bass.AP(tensor=wr.tensor, offset=wr.offset,
                                  ap=[wr.ap[0], [8, 8], [0, 16], [1, 8]])
                    eng = nc.gpsimd if hb % 16 < GPS_WW else nc.vector
                    eng.tensor_tensor(
                        ww.rearrange("p (l o c) -> p l o c", o=16, c=8),
                        wk_block(hb).rearrange("p l (o c) -> p l o c", c=8),
                        in1,
                        op=mybir.AluOpType.mult,
                    )
                    for lo in range(BLK):
                        t = hb * BLK + lo
                        nc.tensor.matmul(ps_n,
                                         lhsT=xt_tile(t),
                                         rhs=ww[:, lo * 128:(lo + 1) * 128],
                                         start=(t == 0), stop=(t == NT - 1))

            # Z^2 per c, replicated to [64, 128] (runs during n-matmuls)
            wsum = small.tile([128, 8], f32, tag="wsum", name="wsum")
            nc.vector.tensor_reduce(
                wsum,
                bass.AP(tensor=wexpb.tensor, offset=wexpb.offset,
                        ap=[wexpb.ap[0], [1, 8], [8, 32]]),
                axis=mybir.AxisListType.X, op=mybir.AluOpType.add,
            )
            ps_z = ps_misc.tile([1, 8], f32, tag="m", name="zps")
            nc.tensor.matmul(ps_z, lhsT=ones128, rhs=wsum, start=True, stop=True)
            zsq = small.tile([1, 8], f32, tag="zsq", name="zsq")
            nc.scalar.activation(zsq, ps_z, mybir.ActivationFunctionType.Square)
            zrow = small.tile([1, 128], f32, tag="zrow", name="zrow")
            nc.vector.tensor_copy(
                zrow.rearrange("p (o c) -> p o c", c=8),
                bass.AP(tensor=zsq.tensor, offset=zsq.offset,
                        ap=[zsq.ap[0], [0, 16], [1, 8]]),
            )
            ps_zq = ps_misc.tile([64, 128], f32, tag="m", name="zqps")
            nc.tensor.matmul(ps_zq, lhsT=onesrow, rhs=zrow, start=True, stop=True)
            zqsb = small.tile([64, 128], f32, tag="zqsb", name="zqsb")
            nc.scalar.activation(zqsb, ps_zq, mybir.ActivationFunctionType.Copy)

            if it < 2:
                V = small.tile([64, 128], cdt, tag="V", name="V", bufs=2)
                squash_from(ps_n, zqsb, V)
            else:
                out_sb = small.tile([64, 128], f32, tag="outsb", name="outsb")
                squash_from(ps_n, zqsb, out_sb)
                nc.sync.dma_start(out=out_d[:], in_=out_sb)

    nc.finalize()
    return nc


def _host_prep(x, W):
    """Build per-core input dicts."""
    import ml_dtypes
    ct = ml_dtypes.bfloat16
    x = np.ascontiguousarray(x, dtype=np.float32)
    W = np.ascontiguousarray(W, dtype=np.float32)
    # xt[p=(q,i), t*64+b] = x[b, 16t+q, i]
    xt = x.reshape(B, NT, 16, I).transpose(2, 3, 1, 0).reshape(128, NT, 64)
    # xn[b, t*128 + q*8+i] = x[b, 16t+q, i]  (natural layout, partitions 0..63)
    xn = x.reshape(B, NT, 128)
    cstb, cstf = _consts_np()
    in_maps = []
    for k in range(N_CORES):
        Ws = W[:, k * CL:(k + 1) * CL]  # [R, 8, O, I]
        wk = (Ws.reshape(NT, 16, CL, O, I).transpose(1, 4, 0, 3, 2)
              .reshape(128, NT, 128))
        wxt = np.zeros((128, 8, 5120), dtype=np.float32)
        for h in range(8):
            wxt[:, h, 0:2048] = wk[:, 16 * h:16 * (h + 1), :].reshape(128, 2048)
            wxt[:, h, 2048:3072] = xt[:, 16 * h:16 * (h + 1), :].reshape(128, 1024)
            wxt[0:64, h, 3072:5120] = xn[:, 16 * h:16 * (h + 1), :].reshape(64, 2048)
        in_maps.append({
            "wxt": np.ascontiguousarray(wxt.reshape(128, 8 * 5120), dtype=ct),
            "cstb": cstb.astype(ct),
            "cstf": cstf,
        })
    return in_maps


_CACHE = {}


def _get_nc():
    if "nc" not in _CACHE:
        _CACHE["nc"] = build_bass()
    return _CACHE["nc"]


def run(x, W, trace=False):
    nc = _get_nc()
    in_maps = _host_prep(x, W)
    res = run_bass_kernel_spmd(nc, in_maps, core_ids=list(range(N_CORES)),
                               trace=trace)
    outs = [np.asarray(res.results[k]["out"], dtype=np.float32)
            for k in range(N_CORES)]
    # out[b, (o, c)]: core k holds capsules [8k, 8k+8)
    v = np.concatenate(
        [o.reshape(B, O, CL).transpose(0, 2, 1) for o in outs], axis=1)
    return v[..., None], res


def kernel(x, W):
    v, _ = run(np.asarray(x), np.asarray(W))
    return v


# revision 5
# speedup vs baseline: 1.0836x; 1.0014x over previous
"""DigitCapsule dynamic-routing kernel for 8 TRN2 NeuronCores.

Strategy: the reference routing is fully independent per output capsule c
(softmax over routes, sums over routes, batch-mean are all per-c). So we
shard the C=64 capsules 8-ways: each core gets W[:, 8k:8k+8] and a
replicated x. Zero collectives; identical SPMD program per core with
per-core inputs.

Per core (B=64, R=2048, I=8, CL=8, O=16; K-dim = (r,i) = 16384 = 128
k-tiles of 128 = (16 routes q, 8 i)). s/v tensors live as
[b=64, (o,c)=128]; routing state lives banded as [(j,q)=128, (g,lo,c)].

  pass 0:  n0[b,(o,c)] = sum_t xt_t^T @ wk_t          (c_ij uniform)
           v = n|n| / (R^2 + n^2)       == squash(n/R), exact algebra
  iter 1,2 (phased so each engine gets long dense runs):
    A: G[(q,i),(lo,(o,c))] = xn^T @ V for all 128 k-tiles — fp8 xn as
       stationary, row-pair tiled (two concurrent 64-row matmuls);
       per block: ACT drains PSUM->bf16, P = G (.) Wr (DVE/GPS)
    B: BD-matmul bands psb[(j,q),(lo,o,c)] per grp; ored = reduce_o;
       bstate += ored/B; wexpb = exp(bstate); wrep matmuls interleaved
    D: WW = Wr (.) wrep (broadcast o);  n += xt_t^T @ WW_t
    Z[c] = sum_r wexp;  v = n|n| / (Z^2 + n^2)  == squash(n/Z), exact
  out[b,(o,c)] = v (f32)
"""

import os
import sys

for _p in ("/opt/trn_rl_repo", "/root/.axon_site/_ro/trn_rl_repo"):
    if os.path.isdir(_p) and _p not in sys.path:
        sys.path.insert(0, _p)

from contextlib import ExitStack

import numpy as np

import concourse.bass as bass
import concourse.bacc as bacc
from concourse import mybir
from concourse.bass_utils import run_bass_kernel_spmd
from concourse.tile import TileContext

B, R, C, O, I = 64, 2048, 64, 16, 8
N_CORES = 8
CL = C // N_CORES            # capsules per core = 8
F = CL * O                   # free (o,c) = 128
NT = R // 16                 # 128 k-tiles; tile t = routes [16t,16t+16), part p=(q,i)
NB = 16                      # number of 8-k-tile blocks
BLK = NT // NB               # 8 k-tiles per block

# which of the 16 P / WW multiplies per iter go to GpSimd instead of DVE
GPS_P = int(os.environ.get("CAPS_GPS_P", "4"))
GPS_WW = int(os.environ.get("CAPS_GPS_WW", "0"))
P_GPS_SET = {3, 7, 11, 15}  # last block of each grp (most slack before BD j=3)
WW_GPS_SET = {15, 14, 13, 12}  # last consumers in the n-matmul sequence


def _consts_np():
    """cstb [128,1024] bf16: BDF4 [0:512), BDT [512:1024).
    cstf [128,65] f32: masked-ones col 0; ones-row (partition 0) cols [1:65)."""
    cstb = np.zeros((128, 1024), dtype=np.float32)
    p = np.arange(128)
    # BDF4_j[p=(q,i), m] = 1 iff m == 32j + p//8  (i-reduce into band 32j+q)
    for j in range(4):
        cstb[p, 128 * j + 32 * j + p // 8] = 1.0
    # BDT_j = BDF4_j^T (band (j,q) -> rows (q,i))
    for j in range(4):
        cstb[:, 512 + 128 * j:512 + 128 * (j + 1)] = \
            cstb[:, 128 * j:128 * (j + 1)].T
    cstf = np.zeros((128, 65), dtype=np.float32)
    # Z-reduce mask: only band rows 32j+q (q<16) hold real data; the other
    # 64 partitions of wexpb are exp(0)=1 junk and must not enter Z.
    cstf[p[(p % 32) < 16], 0] = 1.0
    cstf[0, 1:65] = 1.0
    return cstb, cstf


def build_bass():
    f32 = mybir.dt.float32
    cdt = mybir.dt.bfloat16
    f8 = mybir.dt.float8e4

    nc = bacc.Bacc()
    # wxt: 8 chunks of [wk 2048 | xt 1024] columns
    wxt_d = nc.declare_dram_parameter("wxt", [128, 8 * 3072], cdt, isOutput=False)
    # xn8: fp8 x, natural layout on partitions 0:64
    xn8_d = nc.declare_dram_parameter("xn8", [64, NT * 128], f8, isOutput=False)
    cstb_d = nc.declare_dram_parameter("cstb", [128, 1024], cdt, isOutput=False)
    cstf_d = nc.declare_dram_parameter("cstf", [128, 65], f32, isOutput=False)
    out_d = nc.declare_dram_parameter("out", [B, F], f32, isOutput=True)

    with TileContext(nc) as tc, ExitStack() as ctx:
        big = ctx.enter_context(tc.tile_pool(name="big", bufs=1))
        small = ctx.enter_context(tc.tile_pool(name="small", bufs=3))
        pgpool = ctx.enter_context(tc.tile_pool(name="pgpool", bufs=3))
        p16 = ctx.enter_context(tc.tile_pool(name="p16", bufs=NB + 1))
        wwpool = ctx.enter_context(tc.tile_pool(name="wwpool", bufs=4))
        ps_acc = ctx.enter_context(tc.tile_pool(name="ps_acc", bufs=1, space="PSUM"))
        ps_gb = ctx.enter_context(tc.tile_pool(name="ps_gb", bufs=3, space="PSUM"))
        ps_misc = ctx.enter_context(tc.tile_pool(name="ps_misc", bufs=1, space="PSUM"))

        # ---- load inputs (consts first: small and needed early) ----
        cstb = big.tile([128, 1024], cdt, tag="cstb", name="cstb")
        nc.sync.dma_start(out=cstb, in_=cstb_d[:])
        cstf = big.tile([128, 65], f32, tag="cstf", name="cstf")
        nc.sync.dma_start(out=cstf, in_=cstf_d[:])
        wxt = [big.tile([128, 3072], cdt, tag=f"wxt{h}", name=f"wxt{h}")
               for h in range(8)]
        for h in range(8):
            nc.sync.dma_start(out=wxt[h], in_=wxt_d[:, h * 3072:(h + 1) * 3072])
        xn8 = big.tile([64, NT * 128], f8, tag="xn8", name="xn8")
        for piece in range(2):
            c0 = piece * 8192
            nc.sync.dma_start(out=xn8[:, c0:c0 + 8192],
                              in_=xn8_d[:, c0:c0 + 8192])

        BDF4 = cstb[:, 0:512]
        BDT = cstb[:, 512:1024]
        onesm = cstf[:, 0:1]
        onesrow = cstf[0:1, 1:65]

        def wk_tile(t):
            h, lo = t // 16, t % 16
            return wxt[h][:, lo * 128:(lo + 1) * 128]

        def xt_tile(t):
            h, lo = t // 16, t % 16
            return wxt[h][:, 2048 + lo * 64:2048 + (lo + 1) * 64]

        def wk_block(hb):
            # [128, 8, 128] view of block hb's 8 k-tiles of W
            wkh = wxt[hb // 2][:, 0:2048].rearrange("p (u f) -> p u f", f=128)
            return wkh[:, (hb % 2) * BLK:(hb % 2) * BLK + BLK, :]

        # v = n*|n| / (zsq + n^2); returns V bf16 (mk_V) or out f32
        def squash_from(ps_n, zsq_sb, mk_V):
            absn = small.tile([64, 128], f32, tag="absn", name="absn")
            nc.scalar.activation(absn, ps_n, mybir.ActivationFunctionType.Abs)
            nsq = small.tile([64, 128], f32, tag="nsq", name="nsq")
            nc.scalar.activation(nsq, ps_n, mybir.ActivationFunctionType.Square)
            den = small.tile([64, 128], f32, tag="den", name="den")
            if zsq_sb is None:
                nc.vector.tensor_scalar_add(den, nsq, float(R) * float(R))
            else:
                nc.vector.tensor_add(den, nsq, zsq_sb)
            rden = small.tile([64, 128], f32, tag="rden", name="rden")
            nc.vector.reciprocal(rden, den)
            num = small.tile([64, 128], f32, tag="num", name="num")
            nc.vector.tensor_mul(num, ps_n, absn)
            if not mk_V:
                out_sb = small.tile([64, 128], f32, tag="outsb", name="outsb")
                nc.vector.tensor_mul(out_sb, num, rden)
                return out_sb
            v64 = small.tile([64, 128], cdt, tag="V", name="V", bufs=2)
            nc.vector.tensor_mul(v64, num, rden)
            return v64

        # ---- pass 0: n0 = sum_t xt_t^T @ wk_t ; V = squash ----
        ps_s = ps_acc.tile([64, 128], f32, tag="acc", name="acc")
        for t in range(NT):
            nc.tensor.matmul(ps_s, lhsT=xt_tile(t), rhs=wk_tile(t),
                             start=(t == 0), stop=(t == NT - 1))
        V = squash_from(ps_s, None, True)

        bstate = small.tile([128, 256], f32, tag="bstate", name="bstate", bufs=1)
        nc.vector.memset(bstate, 0.0)
        wexpb = small.tile([128, 256], cdt, tag="wexpb", name="wexpb", bufs=1)

        for it in (1, 2):
            ps_n = ps_acc.tile([64, 128], f32, tag="acc", name="acc")
            Ps = [None] * NB
            # ---------- phase A: all G matmuls (fp8 stationary x) ----------
            for hb in range(NB):
                psg = ps_gb.tile([128, BLK * 128], f32, tag="gb", name="gb")
                for lo in range(BLK):
                    t = hb * BLK + lo
                    nc.tensor.matmul(
                        psg[:, lo * 128:(lo + 1) * 128],
                        lhsT=xn8[:, t * 128:(t + 1) * 128], rhs=V,
                        start=True, stop=True,
                    )
                Pg = pgpool.tile([128, BLK * 128], cdt, tag="Pg", name="Pg")
                nc.scalar.activation(Pg, psg, mybir.ActivationFunctionType.Copy)
                P = p16.tile([128, BLK * 128], cdt, tag="P", name="P")
                eng = nc.gpsimd if (hb in P_GPS_SET and
                                    len(P_GPS_SET) - list(sorted(P_GPS_SET)).index(hb) <= GPS_P) \
                    else nc.vector
                eng.tensor_tensor(
                    P.rearrange("p (u f) -> p u f", f=128),
                    Pg.rearrange("p (u f) -> p u f", f=128),
                    wk_block(hb),
                    op=mybir.AluOpType.mult,
                )
                Ps[hb] = P

            # ---------- phase B: BD bands + b-update + wrep ----------
            psbs = [None] * 4
            wrs = [None] * NB

            def emit_bd(grp):
                psb = ps_gb.tile([128, BLK * 128], f32, tag="gb", name="gb")
                for j in range(4):
                    for half in range(2):
                        nc.tensor.matmul(
                            psb[:, half * 512:(half + 1) * 512],
                            lhsT=BDF4[:, 128 * j:128 * (j + 1)],
                            rhs=Ps[4 * grp + j][:, half * 512:(half + 1) * 512],
                            start=(j == 0), stop=(j == 3),
                        )
                psbs[grp] = psb

            def emit_bupdate(grp):
                ored = small.tile([128, 64], f32, tag="ored", name="ored",
                                  bufs=2)
                psb = psbs[grp]
                nc.vector.tensor_reduce(
                    ored.rearrange("p (l c) -> p l c", c=8),
                    bass.AP(tensor=psb.tensor, offset=psb.offset,
                            ap=[psb.ap[0], [128, 8], [1, 8], [8, 16]]),
                    axis=mybir.AxisListType.X,
                    op=mybir.AluOpType.add,
                )
                cs = slice(grp * 64, (grp + 1) * 64)
                nc.vector.scalar_tensor_tensor(bstate[:, cs], ored, 1.0 / B,
                                               bstate[:, cs],
                                               op0=mybir.AluOpType.mult,
                                               op1=mybir.AluOpType.add)
                nc.scalar.activation(wexpb[:, cs], bstate[:, cs],
                                     mybir.ActivationFunctionType.Exp)

            def emit_wrep(grp):
                cs = slice(grp * 64, (grp + 1) * 64)
                for j in range(4):
                    hb = 4 * grp + j
                    ps_wr = ps_misc.tile([128, 64], f32, tag="m", name="wrps")
                    nc.tensor.matmul(ps_wr, lhsT=BDT[:, 128 * j:128 * (j + 1)],
                                     rhs=wexpb[:, cs], start=True, stop=True)
                    wr = small.tile([128, 64], cdt, tag="wr", name="wr", bufs=5)
                    nc.scalar.activation(wr, ps_wr,
                                         mybir.ActivationFunctionType.Copy)
                    wrs[hb] = wr

            emit_bd(0)
            emit_bupdate(0)
            emit_bd(1)
            emit_bupdate(1)
            emit_wrep(0)
            emit_bd(2)
            emit_bupdate(2)
            emit_wrep(1)
            emit_bd(3)
            emit_bupdate(3)
            emit_wrep(2)
            emit_wrep(3)

            # ---------- phase D: WW + n-matmuls; Z-path in the middle ----------
            def emit_ww_n(hb):
                wr = wrs[hb]
                ww = wwpool.tile([128, BLK * 128], cdt, tag="ww", name="ww")
                in1 = bass.AP(tensor=wr.tensor, offset=wr.offset,
                              ap=[wr.ap[0], [8, 8], [0, 16], [1, 8]])
                eng = nc.gpsimd if (hb in WW_GPS_SET and
                                    list(sorted(WW_GPS_SET, reverse=True)).index(hb) < GPS_WW) \
                    else nc.vector
                eng.tensor_tensor(
                    ww.rearrange("p (l o c) -> p l o c", o=16, c=8),
                    wk_block(hb).rearrange("p l (o c) -> p l o c", c=8),
                    in1,
                    op=mybir.AluOpType.mult,
                )
                for lo in range(BLK):
                    t = hb * BLK + lo
                    nc.tensor.matmul(ps_n,
                                     lhsT=xt_tile(t),
                                     rhs=ww[:, lo * 128:(lo + 1) * 128],
                                     start=(t == 0), stop=(t == NT - 1))

            for hb in range(4):
                emit_ww_n(hb)
            # Z^2 per c, replicated to [64, 128] (overlaps n-matmuls)
            wsum = small.tile([128, 8], f32, tag="wsum", name="wsum")
            nc.vector.tensor_reduce(
                wsum,
                bass.AP(tensor=wexpb.tensor, offset=wexpb.offset,
                        ap=[wexpb.ap[0], [1, 8], [8, 32]]),
                axis=mybir.AxisListType.X, op=mybir.AluOpType.add,
            )
            ps_z = ps_misc.tile([1, 8], f32, tag="m", name="zps")
            nc.tensor.matmul(ps_z, lhsT=onesm, rhs=wsum, start=True, stop=True)
            zsq = small.tile([1, 8], f32, tag="zsq", name="zsq")
            nc.scalar.activation(zsq, ps_z, mybir.ActivationFunctionType.Square)
            zrow = small.tile([1, 128], f32, tag="zrow", name="zrow")
            nc.vector.tensor_copy(
                zrow.rearrange("p (o c) -> p o c", c=8),
                bass.AP(tensor=zsq.tensor, offset=zsq.offset,
                        ap=[zsq.ap[0], [0, 16], [1, 8]]),
            )
            ps_zq = ps_misc.tile([64, 128], f32, tag="m", name="zqps")
            nc.tensor.matmul(ps_zq, lhsT=onesrow, rhs=zrow, start=True, stop=True)
            zqsb = small.tile([64, 128], f32, tag="zqsb", name="zqsb")
            nc.scalar.activation(zqsb, ps_zq, mybir.ActivationFunctionType.Copy)
            for hb in range(4, NB):
                emit_ww_n(hb)

            if it < 2:
                V = squash_from(ps_n, zqsb, True)
            else:
                out_sb = squash_from(ps_n, zqsb, False)
                nc.sync.dma_start(out=out_d[:], in_=out_sb)

    nc.finalize()
    return nc


def _host_prep(x, W):
    """Build per-core input dicts."""
    import ml_dtypes
    ct = ml_dtypes.bfloat16
    f8 = ml_dtypes.float8_e4m3fn
    x = np.ascontiguousarray(x, dtype=np.float32)
    W = np.ascontiguousarray(W, dtype=np.float32)
    # xt[p=(q,i), t*64+b] = x[b, 16t+q, i]
    xt = x.reshape(B, NT, 16, I).transpose(2, 3, 1, 0).reshape(128, NT, 64)
    # xn8[b, t*128 + q*8+i] = x[b, 16t+q, i]  (fp8, partitions 0..63)
    xn8 = x.reshape(B, NT * 128)
    cstb, cstf = _consts_np()
    in_maps = []
    for k in range(N_CORES):
        Ws = W[:, k * CL:(k + 1) * CL]  # [R, 8, O, I]
        wk = (Ws.reshape(NT, 16, CL, O, I).transpose(1, 4, 0, 3, 2)
              .reshape(128, NT, 128))
        wxt = np.zeros((128, 8, 3072), dtype=np.float32)
        for h in range(8):
            wxt[:, h, 0:2048] = wk[:, 16 * h:16 * (h + 1), :].reshape(128, 2048)
            wxt[:, h, 2048:3072] = xt[:, 16 * h:16 * (h + 1), :].reshape(128, 1024)
        in_maps.append({
            "wxt": np.ascontiguousarray(wxt.reshape(128, 8 * 3072), dtype=ct),
            "xn8": xn8.astype(f8),
            "cstb": cstb.astype(ct),
            "cstf": cstf,
        })
    return in_maps


_CACHE = {}


def _get_nc():
    if "nc" not in _CACHE:
        _CACHE["nc"] = build_bass()
    return _CACHE["nc"]


def run(x, W, trace=False):
    nc = _get_nc()
    in_maps = _host_prep(x, W)
    res = run_bass_kernel_spmd(nc, in_maps, core_ids=list(range(N_CORES)),
                               trace=trace)
    outs = [np.asarray(res.results[k]["out"], dtype=np.float32)
            for k in range(N_CORES)]
    # out[b, (o, c)]: core k holds capsules [8k, 8k+8)
    v = np.concatenate(
        [o.reshape(B, O, CL).transpose(0, 2, 1) for o in outs], axis=1)
    return v[..., None], res


def kernel(x, W):
    v, _ = run(np.asarray(x), np.asarray(W))
    return v


# revision 9
# speedup vs baseline: 1.2387x; 1.1431x over previous
"""DigitCapsule dynamic-routing kernel for 8 TRN2 NeuronCores.

Strategy: the reference routing is fully independent per output capsule c
(softmax over routes, sums over routes, batch-mean are all per-c). So we
shard the C=64 capsules 8-ways: each core gets W[:, 8k:8k+8] and a
replicated x. Zero collectives; identical SPMD program per core with
per-core inputs.

Per core (B=64, R=2048, I=8, CL=8, O=16; K-dim = (r,i) = 16384 = 128
k-tiles of 128 = (16 routes q, 8 i)). s/v tensors live as
[b=64, (o,c)=128]; routing state lives banded as [(j,q)=128, (g,lo,c)].

  pass 0:  n0[b,(o,c)] = sum_t xt_t^T @ wk_t          (c_ij uniform)
           v = n|n| / (R^2 + n^2)       == squash(n/R), exact algebra
  iter 1,2 (phased so each engine gets long dense runs):
    A: G[(q,i),(lo,(o,c))] = xn^T @ V for all 128 k-tiles — fp8 xn as
       stationary, row-pair tiled (two concurrent 64-row matmuls);
       per block: ACT drains PSUM->bf16, P = G (.) Wr (DVE/GPS)
    B: BD-matmul bands psb[(j,q),(lo,o,c)] per grp; ored = reduce_o;
       bstate += ored/B; wexpb = exp(bstate); wrep matmuls interleaved
    D: WW = Wr (.) wrep (broadcast o);  n += xt_t^T @ WW_t
    Z[c] = sum_r wexp;  v = n|n| / (Z^2 + n^2)  == squash(n/Z), exact
  out[b,(o,c)] = v (f32)
"""

import os
import sys

for _p in ("/opt/trn_rl_repo", "/root/.axon_site/_ro/trn_rl_repo"):
    if os.path.isdir(_p) and _p not in sys.path:
        sys.path.insert(0, _p)

from contextlib import ExitStack

import numpy as np

import concourse.bass as bass
import concourse.bacc as bacc
from concourse import mybir
from concourse.bass_utils import run_bass_kernel_spmd
from concourse.tile import TileContext

B, R, C, O, I = 64, 2048, 64, 16, 8
N_CORES = 8
CL = C // N_CORES            # capsules per core = 8
F = CL * O                   # free (o,c) = 128
NT = R // 16                 # 128 k-tiles; tile t = routes [16t,16t+16), part p=(q,i)
NB = 16                      # number of 8-k-tile blocks
BLK = NT // NB               # 8 k-tiles per block

# which of the 16 P / WW multiplies per iter go to GpSimd instead of DVE
GPS_P = int(os.environ.get("CAPS_GPS_P", "3"))
GPS_WW = int(os.environ.get("CAPS_GPS_WW", "3"))
P_GPS_SET = {3, 7, 11, 15}  # last block of each grp (most slack before BD j=3)
WW_GPS_SET = {15, 14, 13, 12}  # last consumers in the n-matmul sequence


def _consts_np():
    """cstb [128,1024] bf16: BDF4 [0:512), BDT [512:1024).
    cstf [128,65] f32: masked-ones col 0; ones-row (partition 0) cols [1:65)."""
    cstb = np.zeros((128, 1024), dtype=np.float32)
    p = np.arange(128)
    # BDF4_j[p=(q,i), m] = 1 iff m == 32j + p//8  (i-reduce into band 32j+q)
    for j in range(4):
        cstb[p, 128 * j + 32 * j + p // 8] = 1.0
    # BDT_j = BDF4_j^T (band (j,q) -> rows (q,i))
    for j in range(4):
        cstb[:, 512 + 128 * j:512 + 128 * (j + 1)] = \
            cstb[:, 128 * j:128 * (j + 1)].T
    cstf = np.zeros((128, 65), dtype=np.float32)
    # Z-reduce mask: only band rows 32j+q (q<16) hold real data; the other
    # 64 partitions of wexpb are exp(0)=1 junk and must not enter Z.
    cstf[p[(p % 32) < 16], 0] = 1.0
    cstf[0, 1:65] = 1.0
    return cstb, cstf


def build_bass():
    f32 = mybir.dt.float32
    cdt = mybir.dt.bfloat16
    f8 = mybir.dt.float8e4

    nc = bacc.Bacc()
    # wxt: 8 chunks of [wk 2048 | xt 1024] columns
    wxt_d = nc.declare_dram_parameter("wxt", [128, 8 * 3072], cdt, isOutput=False)
    # xn8: fp8 x, natural layout on partitions 0:64
    # xn8: fp8 x, natural layout on partitions 0:64
    xn8_d = nc.declare_dram_parameter("xn8", [64, NT * 128], f8, isOutput=False)
    cstb_d = nc.declare_dram_parameter("cstb", [128, 1024], cdt, isOutput=False)
    cstf_d = nc.declare_dram_parameter("cstf", [128, 65], f32, isOutput=False)
    out_d = nc.declare_dram_parameter("out", [B, F], f32, isOutput=True)

    with TileContext(nc) as tc, ExitStack() as ctx:
        big = ctx.enter_context(tc.tile_pool(name="big", bufs=1))
        small = ctx.enter_context(tc.tile_pool(name="small", bufs=3))
        pgpool = ctx.enter_context(tc.tile_pool(name="pgpool", bufs=3))
        p16 = ctx.enter_context(tc.tile_pool(name="p16", bufs=NB + 1))
        wwpool = ctx.enter_context(tc.tile_pool(name="wwpool", bufs=4))
        ps_acc = ctx.enter_context(tc.tile_pool(name="ps_acc", bufs=1, space="PSUM"))
        ps_gb = ctx.enter_context(tc.tile_pool(name="ps_gb", bufs=3, space="PSUM"))
        ps_misc = ctx.enter_context(tc.tile_pool(name="ps_misc", bufs=1, space="PSUM"))

        # ---- load inputs (consts first: small and needed early) ----
        cstb = big.tile([128, 1024], cdt, tag="cstb", name="cstb")
        nc.sync.dma_start(out=cstb, in_=cstb_d[:])
        cstf = big.tile([128, 65], f32, tag="cstf", name="cstf")
        nc.sync.dma_start(out=cstf, in_=cstf_d[:])
        wxt = [big.tile([128, 3072], cdt, tag=f"wxt{h}", name=f"wxt{h}")
               for h in range(8)]
        for h in range(8):
            nc.sync.dma_start(out=wxt[h], in_=wxt_d[:, h * 3072:(h + 1) * 3072])
        xn8 = big.tile([64, NT * 128], f8, tag="xn8", name="xn8")
        for piece in range(2):
            c0 = piece * 8192
            nc.sync.dma_start(out=xn8[:, c0:c0 + 8192],
                              in_=xn8_d[:, c0:c0 + 8192])

        BDF4 = cstb[:, 0:512]
        BDT = cstb[:, 512:1024]
        onesm = cstf[:, 0:1]
        onesrow = cstf[0:1, 1:65]

        def wk_tile(t):
            h, lo = t // 16, t % 16
            return wxt[h][:, lo * 128:(lo + 1) * 128]

        def xt_tile(t):
            h, lo = t // 16, t % 16
            return wxt[h][:, 2048 + lo * 64:2048 + (lo + 1) * 64]

        def wk_block(hb):
            # [128, 8, 128] view of block hb's 8 k-tiles of W
            wkh = wxt[hb // 2][:, 0:2048].rearrange("p (u f) -> p u f", f=128)
            return wkh[:, (hb % 2) * BLK:(hb % 2) * BLK + BLK, :]

        # v = n*|n| / (zsq + n^2); returns V bf16 (mk_V) or out f32
        def squash_from(ps_n, zsq_sb, mk_V):
            absn = small.tile([64, 128], f32, tag="absn", name="absn")
            nc.scalar.activation(absn, ps_n, mybir.ActivationFunctionType.Abs)
            nsq = small.tile([64, 128], f32, tag="nsq", name="nsq")
            nc.scalar.activation(nsq, ps_n, mybir.ActivationFunctionType.Square)
            den = small.tile([64, 128], f32, tag="den", name="den")
            if zsq_sb is None:
                nc.vector.tensor_scalar_add(den, nsq, float(R) * float(R))
            else:
                nc.vector.tensor_add(den, nsq, zsq_sb)
            rden = small.tile([64, 128], f32, tag="rden", name="rden")
            nc.vector.reciprocal(rden, den)
            num = small.tile([64, 128], f32, tag="num", name="num")
            nc.vector.tensor_mul(num, ps_n, absn)
            if not mk_V:
                out_sb = small.tile([64, 128], f32, tag="outsb", name="outsb")
                nc.vector.tensor_mul(out_sb, num, rden)
                return out_sb
            v64 = small.tile([64, 128], cdt, tag="V", name="V", bufs=2)
            nc.vector.tensor_mul(v64, num, rden)
            return v64

        # ---- pass 0: n0 = sum_t xt_t^T @ wk_t ; V = squash ----
        ps_s = ps_acc.tile([64, 128], f32, tag="acc", name="acc")
        for t in range(NT):
            nc.tensor.matmul(ps_s, lhsT=xt_tile(t), rhs=wk_tile(t),
                             start=(t == 0), stop=(t == NT - 1))
        V = squash_from(ps_s, None, True)

        bstate = small.tile([128, 256], f32, tag="bstate", name="bstate", bufs=1)
        nc.vector.memset(bstate, 0.0)
        wexpb = small.tile([128, 256], cdt, tag="wexpb", name="wexpb", bufs=1)

        for it in (1, 2):
            ps_n = ps_acc.tile([64, 128], f32, tag="acc", name="acc")
            Ps = [None] * NB
            # ---------- phase A: all G matmuls (fp8 stationary x) ----------
            for hb in range(NB):
                psg = ps_gb.tile([128, BLK * 128], f32, tag="gb", name="gb")
                for lo in range(BLK):
                    t = hb * BLK + lo
                    nc.tensor.matmul(
                        psg[:, lo * 128:(lo + 1) * 128],
                        lhsT=xn8[:, t * 128:(t + 1) * 128], rhs=V,
                        start=True, stop=True,
                    )
                Pg = pgpool.tile([128, BLK * 128], cdt, tag="Pg", name="Pg")
                nc.scalar.activation(Pg, psg, mybir.ActivationFunctionType.Copy)
                P = p16.tile([128, BLK * 128], cdt, tag="P", name="P")
                eng = nc.gpsimd if (hb in P_GPS_SET and
                                    len(P_GPS_SET) - list(sorted(P_GPS_SET)).index(hb) <= GPS_P) \
                    else nc.vector
                eng.tensor_tensor(
                    P.rearrange("p (u f) -> p u f", f=128),
                    Pg.rearrange("p (u f) -> p u f", f=128),
                    wk_block(hb),
                    op=mybir.AluOpType.mult,
                )
                Ps[hb] = P

            # ---------- phase B: BD bands + b-update + wrep ----------
            psbs = [None] * 4
            wrs = [None] * NB

            def emit_bd(grp):
                psb = ps_gb.tile([128, BLK * 128], f32, tag="gb", name="gb")
                for j in range(4):
                    for half in range(2):
                        nc.tensor.matmul(
                            psb[:, half * 512:(half + 1) * 512],
                            lhsT=BDF4[:, 128 * j:128 * (j + 1)],
                            rhs=Ps[4 * grp + j][:, half * 512:(half + 1) * 512],
                            start=(j == 0), stop=(j == 3),
                        )
                psbs[grp] = psb

            def emit_bupdate(grp):
                ored = small.tile([128, 64], f32, tag="ored", name="ored",
                                  bufs=2)
                psb = psbs[grp]
                nc.vector.tensor_reduce(
                    ored.rearrange("p (l c) -> p l c", c=8),
                    bass.AP(tensor=psb.tensor, offset=psb.offset,
                            ap=[psb.ap[0], [128, 8], [1, 8], [8, 16]]),
                    axis=mybir.AxisListType.X,
                    op=mybir.AluOpType.add,
                )
                cs = slice(grp * 64, (grp + 1) * 64)
                nc.vector.scalar_tensor_tensor(bstate[:, cs], ored, 1.0 / B,
                                               bstate[:, cs],
                                               op0=mybir.AluOpType.mult,
                                               op1=mybir.AluOpType.add)
                nc.scalar.activation(wexpb[:, cs], bstate[:, cs],
                                     mybir.ActivationFunctionType.Exp)

            def emit_wrep(grp):
                cs = slice(grp * 64, (grp + 1) * 64)
                for j in range(4):
                    hb = 4 * grp + j
                    ps_wr = ps_misc.tile([128, 64], f32, tag="m", name="wrps")
                    nc.tensor.matmul(ps_wr, lhsT=BDT[:, 128 * j:128 * (j + 1)],
                                     rhs=wexpb[:, cs], start=True, stop=True)
                    wr = small.tile([128, 64], cdt, tag="wr", name="wr", bufs=5)
                    nc.scalar.activation(wr, ps_wr,
                                         mybir.ActivationFunctionType.Copy)
                    wrs[hb] = wr

            emit_bd(0)
            emit_bupdate(0)
            emit_bd(1)
            emit_bupdate(1)
            emit_wrep(0)
            emit_bd(2)
            emit_bupdate(2)
            emit_wrep(1)
            emit_bd(3)
            emit_bupdate(3)
            emit_wrep(2)
            emit_wrep(3)

            # ---------- phase D: WW + n-matmuls; Z-path in the middle ----------
            def emit_ww_n(hb):
                wr = wrs[hb]
                ww = wwpool.tile([128, BLK * 128], cdt, tag="ww", name="ww")
                in1 = bass.AP(tensor=wr.tensor, offset=wr.offset,
                              ap=[wr.ap[0], [8, 8], [0, 16], [1, 8]])
                eng = nc.gpsimd if (hb in WW_GPS_SET and
                                    list(sorted(WW_GPS_SET, reverse=True)).index(hb) < GPS_WW) \
                    else nc.vector
                eng.tensor_tensor(
                    ww.rearrange("p (l o c) -> p l o c", o=16, c=8),
                    wk_block(hb).rearrange("p l (o c) -> p l o c", c=8),
                    in1,
                    op=mybir.AluOpType.mult,
                )
                for lo in range(BLK):
                    t = hb * BLK + lo
                    nc.tensor.matmul(ps_n,
                                     lhsT=xt_tile(t),
                                     rhs=ww[:, lo * 128:(lo + 1) * 128],
                                     start=(t == 0), stop=(t == NT - 1))

            for hb in range(4):
                emit_ww_n(hb)
            # Z^2 per c, replicated to [64, 128] (overlaps n-matmuls)
            wsum = small.tile([128, 8], f32, tag="wsum", name="wsum")
            nc.vector.tensor_reduce(
                wsum,
                bass.AP(tensor=wexpb.tensor, offset=wexpb.offset,
                        ap=[wexpb.ap[0], [1, 8], [8, 32]]),
                axis=mybir.AxisListType.X, op=mybir.AluOpType.add,
            )
            ps_z = ps_misc.tile([1, 8], f32, tag="m", name="zps")
            nc.tensor.matmul(ps_z, lhsT=onesm, rhs=wsum, start=True, stop=True)
            zsq = small.tile([1, 8], f32, tag="zsq", name="zsq")
            nc.scalar.activation(zsq, ps_z, mybir.ActivationFunctionType.Square)
            zrow = small.tile([1, 128], f32, tag="zrow", name="zrow")
            nc.vector.tensor_copy(
                zrow.rearrange("p (o c) -> p o c", c=8),
                bass.AP(tensor=zsq.tensor, offset=zsq.offset,
                        ap=[zsq.ap[0], [0, 16], [1, 8]]),
            )
            ps_zq = ps_misc.tile([64, 128], f32, tag="m", name="zqps")
            nc.tensor.matmul(ps_zq, lhsT=onesrow, rhs=zrow, start=True, stop=True)
            zqsb = small.tile([64, 128], f32, tag="zqsb", name="zqsb")
            nc.scalar.activation(zqsb, ps_zq, mybir.ActivationFunctionType.Copy)
            for hb in range(4, NB):
                emit_ww_n(hb)

            if it < 2:
                V = squash_from(ps_n, zqsb, True)
            else:
                out_sb = squash_from(ps_n, zqsb, False)
                nc.sync.dma_start(out=out_d[:], in_=out_sb)

    nc.finalize()
    return nc


def _host_prep(x, W):
    """Build per-core input dicts."""
    import ml_dtypes
    ct = ml_dtypes.bfloat16
    f8 = ml_dtypes.float8_e4m3fn
    x = np.ascontiguousarray(x, dtype=np.float32)
    W = np.ascontiguousarray(W, dtype=np.float32)
    # xt[p=(q,i), t*64+b] = x[b, 16t+q, i]
    xt = x.reshape(B, NT, 16, I).transpose(2, 3, 1, 0).reshape(128, NT, 64)
    # xn8[b, t*128 + q*8+i] = x[b, 16t+q, i]  (fp8, partitions 0..63)
    xn8 = x.reshape(B, NT * 128)
    cstb, cstf = _consts_np()
    in_maps = []
    for k in range(N_CORES):
        Ws = W[:, k * CL:(k + 1) * CL]  # [R, 8, O, I]
        wk = (Ws.reshape(NT, 16, CL, O, I).transpose(1, 4, 0, 3, 2)
              .reshape(128, NT, 128))
        wxt = np.zeros((128, 8, 3072), dtype=np.float32)
        for h in range(8):
            wxt[:, h, 0:2048] = wk[:, 16 * h:16 * (h + 1), :].reshape(128, 2048)
            wxt[:, h, 2048:3072] = xt[:, 16 * h:16 * (h + 1), :].reshape(128, 1024)
        in_maps.append({
            "wxt": np.ascontiguousarray(wxt.reshape(128, 8 * 3072), dtype=ct),
            "xn8": xn8.astype(f8),
            "cstb": cstb.astype(ct),
            "cstf": cstf,
        })
    return in_maps


_CACHE = {}


def _get_nc():
    if "nc" not in _CACHE:
        _CACHE["nc"] = build_bass()
    return _CACHE["nc"]


def run(x, W, trace=False):
    nc = _get_nc()
    in_maps = _host_prep(x, W)
    res = run_bass_kernel_spmd(nc, in_maps, core_ids=list(range(N_CORES)),
                               trace=trace)
    outs = [np.asarray(res.results[k]["out"], dtype=np.float32)
            for k in range(N_CORES)]
    # out[b, (o, c)]: core k holds capsules [8k, 8k+8)
    v = np.concatenate(
        [o.reshape(B, O, CL).transpose(0, 2, 1) for o in outs], axis=1)
    return v[..., None], res


def kernel(x, W):
    v, _ = run(np.asarray(x), np.asarray(W))
    return v
